# revision 7
# baseline (speedup 1.0000x reference)
"""Trainium2 Bass kernel for nn_ClassicMHA (dense transformer block, linear attention).

Sharding: data-parallel over batch B=8 across the 8 NeuronCores (one batch
element per core, no collectives).

Per-core dataflow (channels-major (C, N) everywhere, N=4096 tokens):
  pass 1: K,V token-major projections per 512-token tile; dot_h = K_h^T V_h
          accumulated in 4 persistent PSUM banks (head pairs, block-diagonal).
  softmax on the 8 (64,64) head dots -> block-diag attn lhsT tiles.
  pass 2: QT -> yT = attn^T QT -> mha = Wo^T yT (Wo,bo pre-doubled on host to
          fold the reference's ln(y+y)) -> LN1 -> z1 = relu(W1^T ln1 + b1)
          -> z2 = W2^T z1 + b2 -> LN2(ln1 + z2) -> out.
  LayerNorm over channels (= partitions) uses ones-vector colsum matmuls for
  stats and K=1 outer-product matmuls to broadcast per-token scalars.

All matmuls run in float32r (TF32-ish, 1 cycle/row at N>=256) with fp32 PSUM
accumulation; fp32r operands are produced by DMA-bitcast or fp32r-typed
engine outputs to satisfy the BIR verifier's rounding rule.
"""

import contextlib
import ctypes
import os
import sys
import types

import numpy as np

# ---------------------------------------------------------------------------
# environment setup: jax persistent compile cache + ntff profile hook shim
# ---------------------------------------------------------------------------

def _setup_env():
    try:
        import jax
        cache_dir = os.environ.get("BASS_JAX_CACHE", "/root/jaxcache")
        os.makedirs(cache_dir, exist_ok=True)
        jax.config.update("jax_compilation_cache_dir", cache_dir)
        jax.config.update("jax_persistent_cache_min_entry_size_bytes", -1)
        jax.config.update("jax_persistent_cache_min_compile_time_secs", 0)
    except Exception:
        pass

    try:
        from antenv.axon_hooks import get_axon_ntff_profile_hook  # noqa: F401
        return
    except ImportError:
        pass
    mod = types.ModuleType("antenv.axon_hooks")
    _holder = {}
    mod.set_axon_ntff_profile_hook = lambda h: _holder.__setitem__("h", h)
    mod.get_axon_ntff_profile_hook = lambda: _holder.get("h")
    sys.modules["antenv.axon_hooks"] = mod
    try:
        import antenv
        antenv.axon_hooks = mod
    except ImportError:
        pass
    try:
        lib = ctypes.CDLL("/opt/axon/libaxon_pjrt.so")
        if not hasattr(lib, "axon_start_nrt_profile"):
            return
        lib.axon_start_nrt_profile.argtypes = [ctypes.POINTER(ctypes.c_int64), ctypes.c_size_t]
        lib.axon_start_nrt_profile.restype = ctypes.c_int64
        lib.axon_stop_nrt_profile.argtypes = [ctypes.c_char_p]
        lib.axon_stop_nrt_profile.restype = ctypes.c_int64

        @contextlib.contextmanager
        def _hook(output_dir, device_ids):
            import jax
            jax.devices()
            if device_ids:
                ids = (ctypes.c_int64 * len(device_ids))(*device_ids)
                rc = lib.axon_start_nrt_profile(ids, len(device_ids))
            else:
                rc = lib.axon_start_nrt_profile(None, 0)
            if rc != 0:
                raise RuntimeError(f"axon_start_nrt_profile rc={rc}")
            try:
                yield
            finally:
                n = lib.axon_stop_nrt_profile(str(output_dir).encode())
                print(f"profile: {n} file(s) -> {output_dir}", file=sys.stderr)

        mod.set_axon_ntff_profile_hook(_hook)
    except Exception:
        pass


_setup_env()

import concourse.bass as bass  # noqa: E402
import concourse.tile as tile  # noqa: E402
from concourse import bacc, mybir  # noqa: E402
from concourse.bass_utils import run_bass_kernel_spmd  # noqa: E402

f32 = mybir.dt.float32
f32r = mybir.dt.float32r
AF = mybir.ActivationFunctionType
Alu = mybir.AluOpType
AX = mybir.AxisListType

B, D, N, H, HD = 8, 512, 4096, 8, 64
FF = 4 * D            # 2048
T = 512               # tokens per n-tile
NT = N // T           # 8
KD = D // 128         # 4 k-tiles over model dim
MD = D // 128         # 4 m-tiles over model dim
FM = FF // 128        # 16 m-tiles over ffn dim
PAIRS = H // 2        # 4 head pairs (2x64 channels)
EPS = 1e-5


def build_nc():
    nc = bacc.Bacc("TRN2", target_bir_lowering=False, debug=False)

    x_d = nc.dram_tensor("x", [D, N], f32, kind="ExternalInput")
    wq_d = nc.dram_tensor("wq", [D, D], f32, kind="ExternalInput")
    wk_d = nc.dram_tensor("wk", [D, D], f32, kind="ExternalInput")
    wv_d = nc.dram_tensor("wv", [D, D], f32, kind="ExternalInput")
    wo_d = nc.dram_tensor("wo", [D, D], f32, kind="ExternalInput")
    w1_d = nc.dram_tensor("w1", [D, FF], f32, kind="ExternalInput")
    w2_d = nc.dram_tensor("w2", [FF, D], f32, kind="ExternalInput")
    bq_d = nc.dram_tensor("bq", [D], f32, kind="ExternalInput")
    bk_d = nc.dram_tensor("bk", [D], f32, kind="ExternalInput")
    bv_d = nc.dram_tensor("bv", [D], f32, kind="ExternalInput")
    bo_d = nc.dram_tensor("bo", [D], f32, kind="ExternalInput")
    b1_d = nc.dram_tensor("b1", [FF], f32, kind="ExternalInput")
    b2_d = nc.dram_tensor("b2", [D], f32, kind="ExternalInput")
    g1_d = nc.dram_tensor("g1", [D], f32, kind="ExternalInput")
    be1_d = nc.dram_tensor("be1", [D], f32, kind="ExternalInput")
    g2_d = nc.dram_tensor("g2", [D], f32, kind="ExternalInput")
    be2_d = nc.dram_tensor("be2", [D], f32, kind="ExternalInput")
    out_d = nc.dram_tensor("out", [D, N], f32, kind="ExternalOutput")

    col = lambda d: d.ap().rearrange("(p o) -> p o", o=1)
    row = lambda d: d.ap().rearrange("(o f) -> o f", o=1)

    with tile.TileContext(nc) as tc, contextlib.ExitStack() as top:
        wp = top.enter_context(tc.tile_pool(name="wts", bufs=1))
        xp = top.enter_context(tc.tile_pool(name="xp", bufs=8))
        rows = top.enter_context(tc.tile_pool(name="rows", bufs=5))
        smalls = top.enter_context(tc.tile_pool(name="smalls", bufs=8))

        # ---- resident weights (fp32r via DMA bitcast) ----
        def load_w(dram, nk, ncols, tag):
            ts = []
            for k in range(nk):
                t_ = wp.tile([128, ncols], f32r, tag=f"{tag}{k}")
                nc.sync.dma_start(
                    t_[:], dram.ap()[k * 128:(k + 1) * 128, :].bitcast(f32r))
                ts.append(t_)
            return ts

        WQ = load_w(wq_d, KD, D, "wq")
        WK = load_w(wk_d, KD, D, "wk")
        WV = load_w(wv_d, KD, D, "wv")
        WO = load_w(wo_d, KD, D, "wo")
        W1 = load_w(w1_d, KD, FF, "w1")

        def load_cols(dram, nm, tag):
            ts = []
            for m in range(nm):
                t_ = wp.tile([128, 1], f32, tag=f"{tag}{m}")
                nc.sync.dma_start(t_[:], col(dram)[m * 128:(m + 1) * 128, :])
                ts.append(t_)
            return ts

        BQ = load_cols(bq_d, MD, "bq")
        BO2 = load_cols(bo_d, MD, "bo")
        B1C = load_cols(b1_d, FM, "b1")
        B2C = load_cols(b2_d, MD, "b2")
        G1 = load_cols(g1_d, MD, "g1")
        BE1 = load_cols(be1_d, MD, "be1")
        G2 = load_cols(g2_d, MD, "g2")
        BE2 = load_cols(be2_d, MD, "be2")

        # constants: ones column (colsum lhsT), ones row (broadcast lhsT)
        ones_c32 = wp.tile([128, 1], f32, tag="onc32")
        nc.vector.memset(ones_c32[:], 1.0)
        ones_c = wp.tile([128, 1], f32r, tag="onc")
        nc.vector.tensor_copy(ones_c[:], ones_c32[:])
        ones_r32 = wp.tile([1, 128], f32, tag="onr32")
        nc.vector.memset(ones_r32[:], 1.0)
        ones_r = wp.tile([1, 128], f32r, tag="onr")
        nc.vector.tensor_copy(ones_r[:], ones_r32[:])
        eps_c = wp.tile([1, 1], f32, tag="epsc")
        nc.vector.memset(eps_c[:], EPS)

        # bk/bv rows (fp32r for broadcast matmul rhs)
        bk_r = wp.tile([1, D], f32r, tag="bkr")
        nc.sync.dma_start(bk_r[:], row(bk_d).bitcast(f32r))
        bv_r = wp.tile([1, D], f32r, tag="bvr")
        nc.sync.dma_start(bv_r[:], row(bv_d).bitcast(f32r))
        BKB = wp.tile([128, D], f32, tag="bkb")
        BVB = wp.tile([128, D], f32, tag="bvb")

        # block-diag attn lhsT tiles (filled after pass 1)
        BD = [wp.tile([128, 128], f32r, tag=f"bd{p}", name=f"bd{p}") for p in range(PAIRS)]

        # =============================== pass 1 ===============================
        with tc.tile_pool(name="kv", bufs=5) as kvp, \
             tc.tile_pool(name="dps", bufs=1, space="PSUM") as dps, \
             tc.tile_pool(name="kps", bufs=4, space="PSUM") as kps:

            # one-time broadcast of bk/bv rows across partitions
            for src, dst in ((bk_r, BKB), (bv_r, BVB)):
                ps = kps.tile([128, D], f32, tag="kvps")
                nc.tensor.matmul(ps[:], ones_r[:], src[:], start=True, stop=True)
                nc.vector.tensor_copy(dst[:], ps[:])

            dot = [dps.tile([128, 128], f32, tag=f"dot{p}", name=f"dot{p}") for p in range(PAIRS)]

            for t in range(NT):
                xt = []
                for k in range(KD):
                    x_t = xp.tile([128, T], f32r, tag="x")
                    nc.sync.dma_start(
                        x_t[:],
                        x_d.ap()[k * 128:(k + 1) * 128,
                                 t * T:(t + 1) * T].bitcast(f32r))
                    xt.append(x_t)
                for st in range(T // 128):
                    xs = [x_t[:, st * 128:(st + 1) * 128] for x_t in xt]
                    kps_t = kps.tile([128, D], f32, tag="kvps")
                    for k in range(KD):
                        nc.tensor.matmul(kps_t[:], xs[k], WK[k][:],
                                         start=(k == 0), stop=(k == KD - 1))
                    ksb = kvp.tile([128, D], f32r, tag="ksb")
                    nc.vector.tensor_tensor(ksb[:], kps_t[:], BKB[:], op=Alu.add)

                    vps_t = kps.tile([128, D], f32, tag="kvps")
                    for k in range(KD):
                        nc.tensor.matmul(vps_t[:], xs[k], WV[k][:],
                                         start=(k == 0), stop=(k == KD - 1))
                    vsb = kvp.tile([128, D], f32r, tag="vsb")
                    nc.vector.tensor_tensor(vsb[:], vps_t[:], BVB[:], op=Alu.add)

                    first = (t == 0 and st == 0)
                    last = (t == NT - 1 and st == T // 128 - 1)
                    for p in range(PAIRS):
                        nc.tensor.matmul(
                            dot[p][:],
                            ksb[:, p * 128:(p + 1) * 128],
                            vsb[:, p * 128:(p + 1) * 128],
                            start=first, stop=last, skip_group_check=True)

            # ---- softmax over e (free axis) per 64x64 head block ----
            for p in range(PAIRS):
                S = smalls.tile([128, 128], f32, tag="sm_s")
                nc.scalar.activation(S[:], dot[p][:], AF.Copy, scale=1.0 / 8.0)
                nm = smalls.tile([128, 1], f32, tag="sm_nm")
                E = smalls.tile([128, 128], f32, tag="sm_e")
                se = smalls.tile([128, 1], f32, tag="sm_se")
                ri = smalls.tile([128, 1], f32, tag="sm_ri")
                for h0 in (0, 64):
                    blk = slice(h0, h0 + 64)
                    nc.vector.reduce_max(nm[blk], S[blk, blk], axis=AX.X,
                                         negate=True)
                    nc.scalar.activation(E[blk, blk], S[blk, blk], AF.Exp,
                                         bias=nm[blk], accum_out=se[blk])
                nc.vector.reciprocal(ri[:], se[:])
                bd = BD[p]
                nc.vector.tensor_scalar_mul(bd[0:64, 0:64], E[0:64, 0:64], ri[0:64])
                nc.vector.tensor_scalar_mul(bd[64:128, 64:128], E[64:128, 64:128],
                                            ri[64:128])
                nc.vector.tensor_scalar_mul(bd[0:64, 64:128], S[0:64, 64:128], 0.0)
                nc.vector.tensor_scalar_mul(bd[64:128, 0:64], S[64:128, 0:64], 0.0)

        # =============================== pass 2 ===============================
        with tc.tile_pool(name="act", bufs=9) as actp, \
             tc.tile_pool(name="mh", bufs=5) as mhp, \
             tc.tile_pool(name="scr", bufs=2) as scp, \
             tc.tile_pool(name="lnp", bufs=5) as lnp, \
             tc.tile_pool(name="z1p", bufs=4) as z1p, \
             tc.tile_pool(name="w2p", bufs=6) as w2p, \
             tc.tile_pool(name="outp", bufs=3) as outp, \
             tc.tile_pool(name="mmps", bufs=2, space="PSUM") as mmps, \
             tc.tile_pool(name="z2ps", bufs=4, space="PSUM") as z2ps, \
             tc.tile_pool(name="bcps", bufs=2, space="PSUM") as bcps:

            def emit_ln(SRC, G, BE, out_pool, out_tag, out_dtype):
                """LayerNorm over channels of 4 x (128, T) fp32r tiles."""
                st_s = mmps.tile([1, T], f32, tag="mm", name="lnsum")
                for m in range(MD):
                    nc.tensor.matmul(st_s[:], ones_c[:], SRC[m][:],
                                     start=(m == 0), stop=(m == MD - 1))
                sqs = []
                for m in range(MD):
                    sq = scp.tile([128, T], f32r, tag="sq")
                    src32 = SRC[m][:].bitcast(f32)
                    nc.vector.tensor_tensor(sq[:], src32, src32, op=Alu.mult)
                    sqs.append(sq)
                st_ss = mmps.tile([1, T], f32, tag="mm", name="lnsumsq")
                for m in range(MD):
                    nc.tensor.matmul(st_ss[:], ones_c[:], sqs[m][:],
                                     start=(m == 0), stop=(m == MD - 1))
                r_mneg = rows.tile([1, T], f32, tag="row")
                nc.vector.tensor_scalar_mul(r_mneg[:], st_s[:], -1.0 / D)
                r_var = rows.tile([1, T], f32, tag="row")
                nc.vector.tensor_scalar_mul(r_var[:], st_ss[:], 1.0 / D)
                r_m2 = rows.tile([1, T], f32, tag="row")
                nc.vector.tensor_mul(r_m2[:], r_mneg[:], r_mneg[:])
                nc.vector.tensor_sub(r_var[:], r_var[:], r_m2[:])
                r_sd = rows.tile([1, T], f32, tag="row")
                nc.scalar.activation(r_sd[:], r_var[:], AF.Sqrt, bias=eps_c[:])
                nc.vector.reciprocal(r_var[:], r_sd[:])      # r_var := rstd
                r_rstd = rows.tile([1, T], f32r, tag="rowr", bufs=2)
                nc.vector.tensor_copy(r_rstd[:], r_var[:])
                nc.vector.tensor_mul(r_mneg[:], r_mneg[:], r_var[:])  # := bneg
                r_bneg = rows.tile([1, T], f32r, tag="rowr", bufs=2)
                nc.vector.tensor_copy(r_bneg[:], r_mneg[:])

                R = bcps.tile([128, T], f32, tag="bc")
                nc.tensor.matmul(R[:], ones_r[:], r_rstd[:], start=True, stop=True)
                t1s = []
                for m in range(MD):
                    t1 = scp.tile([128, T], f32, tag="t1")
                    nc.vector.tensor_tensor(t1[:], SRC[m][:].bitcast(f32), R[:],
                                            op=Alu.mult)
                    t1s.append(t1)
                Bn = bcps.tile([128, T], f32, tag="bc")
                nc.tensor.matmul(Bn[:], ones_r[:], r_bneg[:], start=True, stop=True)
                outs = []
                for m in range(MD):
                    t2 = t1s[m]
                    nc.vector.tensor_tensor(t2[:], t1s[m][:], Bn[:], op=Alu.add)
                    o = out_pool.tile([128, T], out_dtype, tag=out_tag)
                    nc.gpsimd.tensor_scalar(o[:], t2[:], G[m][:], BE[m][:],
                                            op0=Alu.mult, op1=Alu.add)
                    outs.append(o)
                return outs

            for t in range(NT):
                xt = []
                for k in range(KD):
                    x_t = xp.tile([128, T], f32r, tag="x")
                    nc.sync.dma_start(
                        x_t[:],
                        x_d.ap()[k * 128:(k + 1) * 128,
                                 t * T:(t + 1) * T].bitcast(f32r))
                    xt.append(x_t)

                # QT (channels-major)
                QT = []
                for m in range(MD):
                    ps = mmps.tile([128, T], f32, tag="mm")
                    for k in range(KD):
                        nc.tensor.matmul(ps[:], WQ[k][:, m * 128:(m + 1) * 128],
                                         xt[k][:], start=(k == 0),
                                         stop=(k == KD - 1))
                    qt = actp.tile([128, T], f32r, tag="qy")
                    nc.vector.tensor_scalar_add(qt[:], ps[:], BQ[m][:])
                    QT.append(qt)

                # attention apply: yT = BD^T @ QT per head pair
                YT = []
                for p in range(PAIRS):
                    ps = mmps.tile([128, T], f32, tag="mm")
                    nc.tensor.matmul(ps[:], BD[p][:], QT[p][:], start=True,
                                     stop=True)
                    yt = actp.tile([128, T], f32r, tag="qy")
                    nc.vector.tensor_copy(yt[:], ps[:])
                    YT.append(yt)

                # mha2 = (2Wo)^T yT + 2bo
                MH = []
                for m in range(MD):
                    ps = mmps.tile([128, T], f32, tag="mm")
                    for k in range(KD):
                        nc.tensor.matmul(ps[:], WO[k][:, m * 128:(m + 1) * 128],
                                         YT[k][:], start=(k == 0),
                                         stop=(k == KD - 1))
                    mh = mhp.tile([128, T], f32r, tag="mh")
                    nc.vector.tensor_scalar_add(mh[:], ps[:], BO2[m][:])
                    MH.append(mh)

                LN1 = emit_ln(MH, G1, BE1, lnp, "ln1", f32r)

                # z1 = relu(W1^T ln1 + b1); z2 accumulation interleaved
                # (1-step software pipeline keeps PE dense and z1 bufs small)
                zps = [z2ps.tile([128, T], f32, tag="z2", name=f"z2_{t}_{m}") for m in range(MD)]
                Z1 = [None] * FM

                def emit_z1(fm):
                    ps = mmps.tile([128, T], f32, tag="mm", name=f"z1ps_{t}_{fm}")
                    for k in range(KD):
                        nc.tensor.matmul(ps[:], W1[k][:, fm * 128:(fm + 1) * 128],
                                         LN1[k][:], start=(k == 0),
                                         stop=(k == KD - 1))
                    z1 = z1p.tile([128, T], f32r, tag="z1", name=f"z1_{t}_{fm}")
                    nc.scalar.activation(z1[:], ps[:], AF.Relu, bias=B1C[fm][:])
                    Z1[fm] = z1

                def emit_z2(fk):
                    w2t = w2p.tile([128, D], f32r, tag="w2", name=f"w2_{t}_{fk}")
                    nc.gpsimd.dma_start(
                        w2t[:],
                        w2_d.ap()[fk * 128:(fk + 1) * 128, :].bitcast(f32r))
                    for m in range(MD):
                        nc.tensor.matmul(zps[m][:],
                                         w2t[:, m * 128:(m + 1) * 128],
                                         Z1[fk][:], start=(fk == 0),
                                         stop=(fk == FM - 1))

                for fm in range(FM):
                    emit_z1(fm)
                    if fm >= 1:
                        emit_z2(fm - 1)
                emit_z2(FM - 1)
                SR = []
                for m in range(MD):
                    s0 = scp.tile([128, T], f32, tag="s0")
                    nc.vector.tensor_scalar_add(s0[:], zps[m][:], B2C[m][:])
                    sr = mhp.tile([128, T], f32r, tag="mh")
                    nc.vector.tensor_tensor(sr[:], s0[:], LN1[m][:].bitcast(f32),
                                            op=Alu.add)
                    SR.append(sr)

                OUT = emit_ln(SR, G2, BE2, outp, "out", f32)
                for m in range(MD):
                    nc.gpsimd.dma_start(
                        out_d.ap()[m * 128:(m + 1) * 128, t * T:(t + 1) * T],
                        OUT[m][:])

    nc.compile()
    return nc


_NC = None


def _get_nc():
    global _NC
    if _NC is None:
        _NC = build_nc()
    return _NC


def kernel(x, Wq, bq, Wk, bk, Wv, bv, Wo, bo, W1, b1, W2, b2, g1, be1, g2, be2):
    nc = _get_nc()
    a = lambda v: np.ascontiguousarray(np.asarray(v, dtype=np.float32))
    x = a(x)
    shared = {
        "wq": a(Wq), "wk": a(Wk), "wv": a(Wv), "wo": a(Wo) * 2.0,
        "w1": a(W1), "w2": a(W2),
        "bq": a(bq), "bk": a(bk), "bv": a(bv), "bo": a(bo) * 2.0,
        "b1": a(b1), "b2": a(b2),
        "g1": a(g1), "be1": a(be1), "g2": a(g2), "be2": a(be2),
    }
    in_maps = [{"x": np.ascontiguousarray(x[b]), **shared} for b in range(B)]
    res = run_bass_kernel_spmd(nc, in_maps, list(range(B)))
    return np.stack([res.results[b]["out"] for b in range(B)], axis=0)
